# revision 1
# baseline (speedup 1.0000x reference)
"""SphericalConv (gather-based 3x3 conv + 2x nearest upsample) on 8 trn2 cores.

Strategy (data-parallel over batch, one batch image per core):
  1. S_k = sum_c w[c,k] * F[c,:,:] for the 9 taps, via PE matmuls with a
     block-diagonal stationary [128, 32] (two source rows x 9 taps, zero-padded
     to 16-aligned tap slots).  Streamed over 256 source rows.
  2. S rows are written (doubled: [row|row]) to a DRAM scratch S2X so that a
     circular shift of a row is one contiguous 512-float read.
  3. The spherical gather out[h,w] = sum_k S_k[gi(h,k), (w+d(h,k)) mod W] is
     18 indirect DMA gathers (one per (h-parity, tap)): per-partition element
     offsets pick row+shift.  Offsets are computed on the host from gi/gj.
  4. 9-tap sum on DVE, nearest-neighbor 2x upsample = strided DVE copies +
     two output DMAs.

The gi/gj maps produced by the gnomonic projection are row-structured
(gi constant along w; gj a per-row circular shift).  This is verified on the
host; arbitrary (unstructured) index maps fall back to a host computation.
"""

import sys

sys.path.insert(0, "/opt/trn_rl_repo")

import numpy as np

B, C, H, W = 8, 64, 256, 512
NCORES = 8
TAPS = 9
ROWLEN = 1024  # doubled S row
NROWS_X = 4096  # S2X rows incl. 16-aligned tap padding (trash)
NTOT = NROWS_X * ROWLEN

_prog_cache = {}


def _split_multi_waits(nc, mybir):
    # This container's walrus rejects >1 sync wait per instruction; hoist the
    # extra waits onto standalone event-semaphore instructions just before.
    n = 0
    for blk in nc.m.functions[0].blocks:
        insts = blk.instructions
        new, changed = [], False
        for i in insts:
            si = i.sync_info
            if si is not None and len(si.on_wait) > 1:
                waits = list(si.on_wait)
                for w in waits[:-1]:
                    n += 1
                    ev = mybir.InstEventSemaphore(
                        name=f"wsplit_{n}_{i.name}",
                        engine=i.engine,
                        sync_info=mybir.SyncInfo(on_wait=[w], on_update=[]),
                    )
                    new.append(ev)
                i.sync_info = mybir.SyncInfo(
                    on_wait=[waits[-1]], on_update=list(si.on_update)
                )
                changed = True
            new.append(i)
        if changed:
            blk.instructions = new


def _build_program(split_waits=True):
    key = "nc" if split_waits else "nc_raw"
    if key in _prog_cache:
        return _prog_cache[key]

    import concourse.bass as bass
    import concourse.tile as tile
    from concourse import mybir
    from concourse.bass import AP, IndirectOffsetOnAxis

    nc = bass.Bass("TRN2", target_bir_lowering=False, debug=False)
    feat = nc.dram_tensor("feat", [C, H, W], mybir.dt.float32, kind="ExternalInput")
    wbd = nc.dram_tensor("wbd", [128, 32], mybir.dt.float32, kind="ExternalInput")
    offs = nc.dram_tensor("offs", [128, 18], mybir.dt.int32, kind="ExternalInput")
    out = nc.dram_tensor("out", [2 * H, 2 * W], mybir.dt.float32, kind="ExternalOutput")
    s2x = nc.dram_tensor("s2x", [NTOT], mybir.dt.float32)  # internal scratch

    with tile.TileContext(nc) as tc:
        with (
            tc.tile_pool(name="consts", bufs=1) as consts,
            tc.tile_pool(name="ft", bufs=3) as ftp,
            tc.tile_pool(name="ps", bufs=4, space="PSUM") as psp,
            tc.tile_pool(name="stage", bufs=2) as stp,
            tc.tile_pool(name="gath", bufs=1) as gap,
            tc.tile_pool(name="outp", bufs=1) as outp,
        ):
            wt = consts.tile([128, 32], mybir.dt.float32)
            nc.sync.dma_start(wt[:], wbd.ap())
            offs_t = consts.tile([128, 18], mybir.dt.int32)
            nc.sync.dma_start(offs_t[:], offs.ap())

            # main loop: 32 groups x 4 row-pairs (rowA=4m+i, rowB=128+4m+i)
            st = None
            for m in range(32):
                ft = ftp.tile([128, 2048], mybir.dt.float32)
                # partitions 0-63 = channels for rows 4m..4m+3,
                # partitions 64-127 = channels for rows 128+4m..128+4m+3
                src = AP(
                    feat, 4 * m * W, [(128 * W, 2), (H * W, C), (1, 4 * W)]
                )
                nc.sync.dma_start(ft[:], src)

                ps = psp.tile([128, 512], mybir.dt.float32)
                for i in range(4):
                    nc.tensor.matmul(
                        ps[32 * i : 32 * i + 32, :],
                        lhsT=wt[:],
                        rhs=ft[:, 512 * i : 512 * (i + 1)],
                        start=True,
                        stop=True,
                        # base_partition auto-derive caps at 64; pass explicitly
                        tile_position=(0, 32 * i),
                    )

                if m % 16 == 0:
                    st = stp.tile([128, 16 * 512], mybir.dt.float32)
                mm = m % 16
                nc.vector.tensor_copy(st[:, mm * 512 : (mm + 1) * 512], ps[:])

                if mm == 15:
                    flush = m // 16
                    for dbl in range(2):
                        dst = AP(
                            s2x,
                            flush * 2048 * ROWLEN + dbl * 512,
                            [(ROWLEN, 2048), (1, 512)],
                        )
                        nc.scalar.dma_start(dst, st[:])

            # gather: 18 slots = (h parity hd, tap k); h = 2p + hd
            ga = gap.tile([128, 2, 9, 512], mybir.dt.float32)
            for hd in range(2):
                for k in range(9):
                    s = hd * 9 + k
                    nc.gpsimd.indirect_dma_start(
                        out=ga[:, hd, k, :],
                        out_offset=None,
                        in_=AP(s2x, 0, [(1, NTOT), (1, 1)]),
                        in_offset=IndirectOffsetOnAxis(ap=offs_t[:, s : s + 1], axis=0),
                    )

            # 9-tap sum -> o [128, 2, 512]
            o = outp.tile([128, 2, 512], mybir.dt.float32)
            nc.vector.tensor_copy(o[:], ga[:, :, 0, :])
            for k in range(1, 9):
                nc.vector.tensor_add(o[:], o[:], ga[:, :, k, :])

            # column-double into o2 [128, 2, 512, 2]
            o2 = outp.tile([128, 2, 512, 2], mybir.dt.float32)
            for bcol in range(2):
                nc.vector.tensor_copy(o2[:, :, :, bcol], o[:])

            # row-double via two output DMAs; out row = 4p + 2*hd + a
            for a in range(2):
                dst = AP(
                    out, a * (2 * W), [(4 * 2 * W, 128), (2 * 2 * W, 2), (1, 2 * W)]
                )
                nc.sync.dma_start(dst, o2[:].opt(keep_dims={0}))

    if split_waits:
        _split_multi_waits(nc, mybir)
    _prog_cache[key] = nc
    return nc


def _structured(gi, gj):
    if not all(np.array_equal(gi[:, :, k], np.broadcast_to(gi[:, :1, k], (H, W))) for k in range(TAPS)):
        return False
    d = (gj - np.arange(W, dtype=np.int64)[None, :, None]) % W
    return all(np.array_equal(d[:, :, k], np.broadcast_to(d[:, :1, k], (H, W))) for k in range(TAPS))


def _host_fallback(feature, weight, gi, gj):
    # correct-but-slow path for arbitrary (non roll-structured) index maps
    wflat = weight.reshape(1, C, TAPS).astype(np.float32)
    outc = np.zeros((B, H, W), np.float32)
    for k in range(TAPS):
        xk = feature[:, :, gi[:, :, k], gj[:, :, k]]
        outc += np.einsum("bchw,c->bhw", xk, wflat[0, :, k])
    up = np.repeat(np.repeat(outc, 2, axis=1), 2, axis=2)
    return up[:, None].astype(np.float32)


def _make_device_inputs(weight, gi, gj):
    # block-diag stationary [128, 32]: wt[64*half + c, 16*t9 + k] = w[c,k] iff half==t9
    w9 = np.asarray(weight, np.float32).reshape(C, TAPS)
    wbd = np.zeros((128, 32), np.float32)
    for t9 in range(2):
        wbd[64 * t9 : 64 * t9 + 64, 16 * t9 : 16 * t9 + 9] = w9

    r = gi[:, 0, :].astype(np.int64)  # [H, 9]
    d = (gj[:, 0, :].astype(np.int64) - 0) % W  # shift per (h, k)

    # S2X row id for source row rr, tap k:
    t9r = r // 128
    rr = r % 128
    i4 = rr % 4
    mm = rr // 4
    fl = mm // 16
    m16 = mm % 16
    row_id = fl * 2048 + ((i4 * 2 + t9r) * 16 + np.arange(TAPS)[None, :]) * 16 + m16
    off_hk = row_id * ROWLEN + d  # [H, 9]

    offs = np.zeros((128, 18), np.int32)
    for hd in range(2):
        for k in range(TAPS):
            offs[:, hd * 9 + k] = off_hk[2 * np.arange(128) + hd, k]
    return wbd, offs


def _run_device(feature, wbd, offs, trace=False, trace_kwargs=None):
    from concourse.bass_utils import run_bass_kernel_spmd

    nc = _build_program()
    in_maps = [
        {"feat": np.ascontiguousarray(feature[b]), "wbd": wbd, "offs": offs}
        for b in range(B)
    ]
    kw = {}
    if trace:
        kw["trace"] = True
        if trace_kwargs:
            kw.update(trace_kwargs)
    return run_bass_kernel_spmd(nc, in_maps, list(range(NCORES)), **kw)


def kernel(feature, weight, gi, gj):
    feature = np.asarray(feature, dtype=np.float32)
    weight = np.asarray(weight, dtype=np.float32)
    gi = np.asarray(gi)
    gj = np.asarray(gj)

    if not _structured(gi, gj):
        return _host_fallback(feature, weight, gi, gj)

    wbd, offs = _make_device_inputs(weight, gi, gj)
    res = _run_device(feature, wbd, offs)
    out = np.stack([res.results[b]["out"] for b in range(B)])
    return out[:, None].astype(np.float32)

